# revision 49
# baseline (speedup 1.0000x reference)
"""MemAttention Trainium2 kernel (8 NeuronCores, SPMD).

Math (see reference):
  q = gelu(query @ Wq.T + bq); k = gelu(key @ Wk.T + bk)        (erf gelu)
  mem = lam*memory + (1-lam)*q                                  (L == S == MAXL here)
  per (batch n, head h):  out = tril(qh @ kh.T) @ memh          (no softmax)
  out = LayerNorm_E(out) * ln_w + ln_b

Sharding: 2-way data-parallel over batch x 4-way tensor-parallel over heads.
Core c (group g = c//4, pos p = c%4) owns batches {2g, 2g+1} and heads
[4p, 4p+4) == E-slice [256p, 256p+256). Each core reads only its two batches'
(host-pre-transposed, bf16) query/key — half the input bytes of pure head
sharding — projects onto its 256-wide weight slice producing qT/kT in
[head*d, token] layout (two 128-partition head-pair blocks), and runs
attention for 2 batches x 2 head-pairs.

Attention uses the chunked linear-attention form (exact reassociation of the
causal masked product):
  A_i = sum_{s < i*C} k[s] (x) mem[s]          (d x d running state per head)
  out[chunk i] = tril(q_i k_i^T) @ mem_i + q_i @ A_i
  A_{i+1} = A_i + k_i^T @ mem_i
which needs O(L*C) score work instead of O(L^2).

LayerNorm is over the full E=1024, sharded 4-ways within each group: each
core contributes per-row (sum x, sum x^2)/1024 of its 256 columns; a 4-rank
AllGather per row-half (replica groups {0..3} and {4..7}) + a local 4-way
sum yields global stats (AllGather costs ~16.6us vs AllReduce's ~28.9us);
each core then normalizes and writes its (L, 2, 256) output block; the host
assembles the 4x4 blocks.

Scheduling notes (engines execute their static order in-order, so emission
order is the schedule):
 - projection row-tiles and the attention chunks they feed are interleaved so
   PE/DVE/ACT work hides under the input DMA stream;
 - per-chunk work is split into a state-independent "prep" (transposes, mem
   blend, masked diagonal scores — fully pipelined) and a short state-carrying
   "attn" part, with two head-pair streams interleaved to hide chain latency;
 - small collectives are latency-dominated and serialize, so stats use one
   AllGather + local sum per row-half (both batches): the half-0 gather
   hides fully under the half-1 groups, the half-1 gather is the only
   exposed collective latency, with half-0's LN applied underneath it;
 - stats are computed per 4-chunk quarter so the final AR isn't gated on a
   long post-attention reduction chain; LN g-loads are hoisted so the two
   quarters' LN chains pipeline across ACT/GPSIMD/DVE.
"""

import numpy as np
import ml_dtypes

import concourse.bass as bass
import concourse.mybir as mybir
import concourse.tile as tile
from concourse.bass_utils import run_bass_kernel_spmd
from concourse.masks import make_identity, make_upper_triangular

# ---------------------------------------------------------------------------
# Workaround: the walrus build in this container accepts only one sync-wait
# per instruction, but the Tile scheduler emits multi-wait Drains. Hoist the
# extra waits onto inserted NoOps (same engine, so execution order preserves
# semantics). Patched into both the native and the axon/PJRT compile paths.
# ---------------------------------------------------------------------------
import orjson

_MAX_WAITS = 1
_patch_done = False


def _split_waits(bir_json: bytes) -> bytes:
    d = orjson.loads(bir_json)
    n = 0
    for f in d.get("functions", []):
        for bb in f.get("blocks", []):
            instructions = bb.get("instructions")
            if not instructions:
                continue
            out = []
            changed = False
            for ins in instructions:
                si = ins.get("sync_info")
                waits = (si or {}).get("on_wait") or []
                if len(waits) > _MAX_WAITS:
                    changed = True
                    extra, keep = waits[:-_MAX_WAITS], waits[-_MAX_WAITS:]
                    for w in extra:
                        n += 1
                        out.append(
                            {
                                "debug": ins.get("debug", 0),
                                "engine": ins["engine"],
                                "ins": [],
                                "name": f"{ins.get('name', 'I')}-ws{n}",
                                "opcode": "NoOp",
                                "outs": [],
                                "sync_info": {"on_update": [], "on_wait": [w]},
                            }
                        )
                    si["on_wait"] = keep
                out.append(ins)
            if changed:
                bb["instructions"] = out
    return orjson.dumps(d)


def _install_patch():
    global _patch_done
    if _patch_done:
        return
    _patch_done = True
    import concourse.bass_utils as bass_utils
    import concourse.bass2jax as bass2jax

    orig = bass_utils.compile_bir_kernel

    def patched(bir_json, tmpdir, neff_name="file.neff"):
        return orig(_split_waits(bir_json), tmpdir, neff_name)

    bass_utils.compile_bir_kernel = patched
    bass2jax.compile_bir_kernel = patched


# ---------------------------------------------------------------------------
# Problem constants (hardcoded per contest contract)
# ---------------------------------------------------------------------------
L = 2048          # query length (== S == MAXL)
N = 4             # batch
E = 1024          # embed dim
H = 16            # heads
D = E // H        # head dim, 64
LAM = 0.001
LN_EPS = 1e-5
NCORES = 8
NGRP = 4           # cores per replica group (head-parallel within group)
NB = 2             # batches per core
NHP = 2            # head-pairs per core (4 heads)
ESL = 256          # per-core E slice
C = 128            # attention chunk
NCH = L // C       # 16 chunks per sequence
ROWSC = NB * L     # 4096 token rows per core, batch-major
KO = E // 128      # 8 contraction chunks
RT = 1024          # projection row-tile (L // 2)

F32 = mybir.dt.float32
BF16 = mybir.dt.bfloat16
AF = mybir.ActivationFunctionType
ALU = mybir.AluOpType


def _bc(ap, count, axis_pos=1):
    """Broadcast an AP by inserting a 0-stride dim of `count` at axis_pos."""
    new = list(ap.ap)
    new.insert(axis_pos, [0, count])
    return bass.AP(tensor=ap.tensor, offset=ap.offset, ap=new)


def build_nc(affine: bool = True) -> bass.Bass:
    nc = bass.Bass()

    # ---- I/O (per core: 2 batches, 256-wide head slice) ----
    xqT = nc.declare_dram_parameter("xqT", [E, ROWSC], BF16, isOutput=False)
    xkT = nc.declare_dram_parameter("xkT", [E, ROWSC], BF16, isOutput=False)
    wqT = nc.declare_dram_parameter("wqT", [E, ESL], BF16, isOutput=False)
    wkT = nc.declare_dram_parameter("wkT", [E, ESL], BF16, isOutput=False)
    bq = nc.declare_dram_parameter("bq", [ESL], F32, isOutput=False)
    bk = nc.declare_dram_parameter("bk", [ESL], F32, isOutput=False)
    memsc = nc.declare_dram_parameter("memsc", [L, ESL], BF16, isOutput=False)
    lnw = nc.declare_dram_parameter("lnw", [ESL], F32, isOutput=False)
    lnb = nc.declare_dram_parameter("lnb", [ESL], F32, isOutput=False)
    out = nc.declare_dram_parameter("out", [L, NB, ESL], F32, isOutput=True)

    # one stats buffer per ROW-HALF (both batches) -> one AllReduce per half;
    # with groups ordered (0,0),(1,0),(0,1),(1,1) the half-0 AR hides under
    # the half-1 groups and only the half-1 AR's latency is ever exposed
    cc_in = [nc.dram_tensor(f"cc_in{i}", [128, NB * (NCH // 2), 2], F32) for i in range(2)]
    # AllGather output: 4 rank blocks of [128, NCH, 2]; summed locally —
    # the cost model charges AllReduce 1.875x but AllGather only
    # (15us + bytes/40GBps), so gather+local-add halves the collective
    cc_out = [
        nc.dram_tensor(f"cc_out{i}", [NGRP * 128, NB * (NCH // 2), 2], F32)
        for i in range(2)
    ]

    with tile.TileContext(nc) as tc:
        _emit(nc, tc, xqT, xkT, wqT, wkT, bq, bk, memsc, lnw, lnb, out, cc_in, cc_out, affine)
    return nc


def _emit(nc, tc, xqT, xkT, wqT, wkT, bq, bk, memsc, lnw, lnb, out, cc_in, cc_out, affine):
    import contextlib

    ctx = contextlib.ExitStack()
    with ctx:
        singles = ctx.enter_context(tc.tile_pool(name="singles", bufs=1))
        # pools up front (no instructions emitted) so input DMAs can be
        # interleaved with the constant loads in queue order below
        xpool = ctx.enter_context(tc.tile_pool(name="xpool", bufs=4))
        ppool = ctx.enter_context(tc.tile_pool(name="ppool", bufs=2, space="PSUM"))
        tpool = ctx.enter_context(tc.tile_pool(name="tpool", bufs=2, space="PSUM"))
        spool = ctx.enter_context(tc.tile_pool(name="spool", bufs=2, space="PSUM"))
        opool = ctx.enter_context(tc.tile_pool(name="opool", bufs=2, space="PSUM"))
        apool = ctx.enter_context(tc.tile_pool(name="apool", bufs=3))
        stpool = ctx.enter_context(tc.tile_pool(name="stpool", bufs=2))

        def proj_dma(xdram, nl, rh):
            """Queue one row-tile's input stream; split by COLUMN halves so
            the first 512-col matmul group starts after 1MB, not 2MB."""
            r0 = nl * L + rh * RT
            xt = xpool.tile([128, KO, RT], BF16, tag="xt", name="xt")
            xsrc = xdram.rearrange("(ko ki) r -> ki ko r", ki=128)[:, :, r0 : r0 + RT]
            nc.sync.dma_start(out=xt[:, :, 0 : RT // 2], in_=xsrc[:, :, 0 : RT // 2])
            nc.sync.dma_start(out=xt[:, :, RT // 2 :], in_=xsrc[:, :, RT // 2 :])
            return xt

        # ---- DMA queue order: wq -> first xq -> wk -> first xk, so the
        # first matmul group is ready after ~2.5MB of stream instead of ~5MB;
        # mem/lnw/lnb (needed only later) queue after the first inputs ----
        wq_sb = singles.tile([128, KO, ESL], BF16)
        nc.sync.dma_start(out=wq_sb, in_=wqT.rearrange("(ko ki) m -> ki ko m", ki=128))
        # biases are 448B but gate the gelu evacuation (which gates PSUM
        # reuse) — keep them ahead of the megabyte input stream
        bq_sb = singles.tile([128, NHP], F32)
        nc.sync.dma_start(out=bq_sb, in_=bq.rearrange("(hp p) -> p hp", p=128))
        bk_sb = singles.tile([128, NHP], F32)
        nc.sync.dma_start(out=bk_sb, in_=bk.rearrange("(hp p) -> p hp", p=128))
        xtq0 = proj_dma(xqT, 0, 0)
        wk_sb = singles.tile([128, KO, ESL], BF16)
        nc.sync.dma_start(out=wk_sb, in_=wkT.rearrange("(ko ki) m -> ki ko m", ki=128))
        xtk0 = proj_dma(xkT, 0, 0)

        # ---- other constants (loaded while the first projections stream) ----
        ident = singles.tile([128, 128], BF16)
        make_identity(nc, ident)
        cmask = singles.tile([128, 128], F32)  # 1 where s(part) <= l(free)
        make_upper_triangular(nc, cmask, val=1.0, diag=True)
        cmask2 = singles.tile([128, 2, 128], F32)  # duplicated per head
        nc.vector.tensor_copy(cmask2[:, 0, :], cmask)
        nc.vector.tensor_copy(cmask2[:, 1, :], cmask)

        # lam*memory in [s%C, (chunk, e)] layout; first needed by prep(0,*,0)
        mem_sb = singles.tile([128, NCH, ESL], BF16)
        nc.sync.dma_start(out=mem_sb, in_=memsc.rearrange("(c p) e -> p c e", p=128))

        lnw_sb = singles.tile([128, ESL], F32)
        nc.sync.dma_start(out=lnw_sb, in_=_bc(lnw[:], 128, 0))
        lnb_sb = singles.tile([128, ESL], F32)
        nc.sync.dma_start(out=lnb_sb, in_=_bc(lnb[:], 128, 0))
        eps_sb = singles.tile([128, 1], F32)
        nc.vector.memset(eps_sb, LN_EPS)

        # ---- persistent activations: qT/kT per (batch, head-pair, row-half) ----
        qT_t = [
            [[singles.tile([128, RT], BF16, name=f"qT{nl}{hp}{rh}") for rh in range(2)]
             for hp in range(NHP)]
            for nl in range(NB)
        ]
        kT_t = [
            [[singles.tile([128, RT], BF16, name=f"kT{nl}{hp}{rh}") for rh in range(2)]
             for hp in range(NHP)]
            for nl in range(NB)
        ]
        out_sb = singles.tile([128, NB * NCH, ESL], F32)  # [l%C, (nl,ch), e]
        acc_sums = singles.tile([128, NB * NCH, NHP], F32)  # per-chunk/hp sum
        # linear-attention state per (batch, head-pair): [(h,d), s, e_local]
        a32 = singles.tile([128, NB * NHP, D], F32)
        abf = singles.tile([128, NB * NHP, D], BF16)

        def proj_mm(nl, rh, xtq, xtk):
            for xt, w_sb, bias_sb, dst in (
                (xtq, wq_sb, bq_sb, qT_t[nl]),
                (xtk, wk_sb, bk_sb, kT_t[nl]),
            ):
                # st-major: both head-pairs consume column-half st before
                # touching half st+1, matching the column-split DMA arrival
                # order so PE never stalls (and never drops p-state) mid-tile
                for st in range(RT // 512):
                    for hp in range(NHP):
                        ms = slice(hp * 128, (hp + 1) * 128)
                        ps = ppool.tile([128, 512], F32, tag="ps", name="ps")
                        for ko in range(KO):
                            nc.tensor.matmul(
                                ps,
                                w_sb[:, ko, ms],
                                xt[:, ko, st * 512 : (st + 1) * 512],
                                start=(ko == 0),
                                stop=(ko == KO - 1),
                            )
                        nc.scalar.activation(
                            out=dst[hp][rh][:, st * 512 : (st + 1) * 512],
                            in_=ps,
                            func=AF.Gelu,
                            bias=bias_sb[:, hp : hp + 1],
                            scale=1.0,
                        )

        pr_mem = {}
        pr_knat = {}
        pr_st = {}

        def prep(nl, hp, ch):
            """State-independent per-chunk work: transposes, mem blend,
            diagonal scores + causal mask. Fully pipelined, no serial chain."""
            rh, c0 = divmod(ch, 8)
            c0 *= C
            qTs = qT_t[nl][hp][rh][:, c0 : c0 + C]
            kTs = kT_t[nl][hp][rh][:, c0 : c0 + C]
            ms = slice(hp * 128, (hp + 1) * 128)

            # both transposes into one PSUM bank (sequential full-array PE
            # ops; explicit dep guards the half-0 read vs the half-1 write)
            qk_ps = tpool.tile([128, 2, 128], BF16, tag="tp", name="qk_ps")
            nc.tensor.transpose(qk_ps[:, 0, :], qTs, ident)
            i_t1 = nc.tensor.transpose(qk_ps[:, 1, :], kTs, ident)

            mem_nat = apool.tile([128, 128], BF16, tag="mn", bufs=12, name="mem_nat")
            i_stt = nc.vector.scalar_tensor_tensor(
                out=mem_nat,
                in0=qk_ps[:, 0, :],
                scalar=1.0 - LAM,
                in1=mem_sb[:, ch, ms],
                op0=ALU.mult,
                op1=ALU.add,
            )
            tile.add_dep_helper(i_stt.ins, i_t1.ins, reason="qk_ps bank serialize")
            knat = apool.tile([128, 128], BF16, tag="kn", bufs=12, name="knat")
            nc.scalar.copy(knat, qk_ps[:, 1, :])

            # the two heads' transposed scores go to DIFFERENT banks of one
            # 2-bank tile (concurrent row-packed MMs must not share a bank),
            # enabling a single merged causal-mask op
            st2_ps = spool.tile([128, 2, 512], F32, tag="st", bufs=1, name="st2_ps")
            for h in range(2):
                hs = slice(h * D, (h + 1) * D)
                nc.tensor.matmul(
                    st2_ps[:, h, 0:128], kTs[hs, :], qTs[hs, :], start=True, stop=True
                )
            st_sb = apool.tile([128, 2, 128], BF16, tag="stsb", bufs=12, name="st_sb")
            nc.vector.scalar_tensor_tensor(
                out=st_sb,
                in0=st2_ps[:, :, 0:128],
                scalar=1.0,
                in1=cmask2[:, :, :],
                op0=ALU.mult,
                op1=ALU.mult,
            )
            st_sbufs = [st_sb[:, 0, :], st_sb[:, 1, :]]
            pr_mem[(nl, hp, ch)] = mem_nat
            pr_knat[(nl, hp, ch)] = knat
            pr_st[(nl, hp, ch)] = st_sbufs

        def attn(nl, hp, ch):
            """State-carrying part: out = tril(S)@mem + q@A ; A += k^T@mem."""
            rh, c0 = divmod(ch, 8)
            c0 *= C
            slot = nl * NCH + ch
            sid = nl * NHP + hp
            qTs = qT_t[nl][hp][rh][:, c0 : c0 + C]
            mem_nat = pr_mem.pop((nl, hp, ch))
            knat = pr_knat.pop((nl, hp, ch))
            st_sbufs = pr_st.pop((nl, hp, ch))

            op_ps = opool.tile([128, 192], F32, tag="op", name="op_ps")
            for h in range(2):
                hs = slice(h * D, (h + 1) * D)
                nc.tensor.matmul(
                    op_ps[:, h * D : (h + 1) * D],
                    st_sbufs[h],
                    mem_nat[:, hs],
                    start=True,
                    stop=(ch == 0),
                )
                if ch > 0:
                    nc.tensor.matmul(
                        op_ps[:, h * D : (h + 1) * D],
                        qTs[hs, :],
                        abf[hs, sid, :],
                        start=False,
                        stop=True,
                    )
            i_d = None
            for h in range(2):
                hs = slice(h * D, (h + 1) * D)
                i_d = nc.tensor.matmul(
                    op_ps[hs, 128:192],
                    knat[:, hs],
                    mem_nat[:, hs],
                    start=True,
                    stop=True,
                )

            i_ev = nc.scalar.activation(
                out=out_sb[:, slot, hp * 128 : (hp + 1) * 128],
                in_=op_ps[:, 0:128],
                func=AF.Copy,
                accum_out=acc_sums[:, slot, hp : hp + 1],
            )
            # same PSUM bank: don't read cols 0:128 while PE writes 128:192
            tile.add_dep_helper(i_ev.ins, i_d.ins, reason="op_ps bank serialize")
            da = op_ps[:, 128:192]
            if ch == 0:
                nc.vector.tensor_copy(abf[:, sid, :], da)
                nc.vector.tensor_copy(a32[:, sid, :], da)
            else:
                # abf = bf16(a32_old + dA) first (reads old a32), then update
                # the fp32 accumulator; keeps ACT out of the state chain
                nc.vector.scalar_tensor_tensor(
                    out=abf[:, sid, :],
                    in0=da,
                    scalar=1.0,
                    in1=a32[:, sid, :],
                    op0=ALU.mult,
                    op1=ALU.add,
                )
                nc.vector.scalar_tensor_tensor(
                    out=a32[:, sid, :],
                    in0=da,
                    scalar=1.0,
                    in1=a32[:, sid, :],
                    op0=ALU.mult,
                    op1=ALU.add,
                )

        HCH = NCH // 2

        def stats_sq(nl, hf, lo, n):
            # x^2 on Pool only — Pool is not in the attention dependency
            # chain, so the 6-slot piece can run mid-attention without
            # stalling the DVE blends
            s0 = nl * NCH + hf * HCH + lo
            slab = out_sb[:, s0 : s0 + n, :]
            sq = stpool.tile([128, n, ESL], F32, tag=f"sq{n}", name="sq")
            nc.gpsimd.tensor_mul(sq, slab, slab)
            return sq

        def stats_fin(nl, hf, lo, n, sq):
            # mean/4 = sum(x)/1024 (evac accums), E[x^2]/4 = sum(x^2)/1024
            s0 = nl * NCH + hf * HCH + lo
            stats = stpool.tile([128, n, 2], F32, tag=f"stats{n}", name="stats")
            nc.vector.tensor_reduce(
                stats[:, :, 1], sq, axis=mybir.AxisListType.X, op=ALU.add
            )
            acc = acc_sums[:, s0 : s0 + n, :]
            nc.vector.tensor_add(stats[:, :, 0], acc[:, :, 0], acc[:, :, 1])
            nc.scalar.mul(stats, stats, 1.0 / E)
            nc.sync.dma_start(
                out=cc_in[hf][:, nl * HCH + lo : nl * HCH + lo + n, :],
                in_=stats,
            )

        def half_ar(hf):
            # single collective for this row-half of both batches; AllGather
            # + local sum instead of AllReduce: small collectives are
            # latency-dominated and AllReduce costs 1.875x the gather
            nc.gpsimd.collective_compute(
                "AllGather",
                ALU.bypass,
                replica_groups=[[0, 1, 2, 3], [4, 5, 6, 7]],
                ins=[cc_in[hf][:, :, :]],
                outs=[cc_out[hf][:, :, :]],
            )

        def ln_load(nl, hf):
            # fetch all 4 rank blocks and sum locally (order-invariant, so
            # the gather's rank-block layout doesn't matter)
            g4 = stpool.tile([128, NGRP, HCH, 2], F32, tag="g4", name="g4")
            nc.sync.dma_start(
                out=g4,
                in_=cc_out[hf].rearrange("(r p) c s -> p r c s", p=128)[
                    :, :, nl * HCH : (nl + 1) * HCH, :
                ],
            )
            g = stpool.tile([128, HCH, 2], F32, tag="g", name="g")
            nc.vector.tensor_add(g, g4[:, 0], g4[:, 1])
            nc.vector.tensor_add(g, g, g4[:, 2])
            nc.vector.tensor_add(g, g, g4[:, 3])
            return g

        def ln_final(nl, hf, g):
            s0 = nl * NCH + hf * HCH
            mu = g[:, :, 0]
            musq = stpool.tile([128, HCH], F32, tag="musq", name="musq")
            nc.vector.tensor_mul(musq, mu, mu)
            var = stpool.tile([128, HCH], F32, tag="var", name="var")
            nc.vector.tensor_sub(var, g[:, :, 1], musq)
            rstd = stpool.tile([128, HCH], F32, tag="rstd", name="rstd")
            nc.scalar.activation(out=rstd, in_=var, func=AF.Sqrt, bias=eps_sb, scale=1.0)
            nc.vector.reciprocal(rstd, rstd)
            nmr = stpool.tile([128, HCH], F32, tag="nmr", name="nmr")
            nc.vector.scalar_tensor_tensor(
                out=nmr, in0=mu, scalar=-1.0, op0=ALU.mult, in1=rstd, op1=ALU.mult
            )
            odst = out[hf * L // 2 : (hf + 1) * L // 2, nl, :].rearrange(
                "(c p_) e -> p_ c e", p_=128
            )
            for i in range(HCH):
                slot = s0 + i
                # (x - mu) * rstd on ACT, then * ln_w + ln_b; the muls are
                # split between Pool and DVE (Pool's ~600ns/op would other-
                # wise serialize the tail), adds stay on DVE
                nc.scalar.activation(
                    out=out_sb[:, slot, :],
                    in_=out_sb[:, slot, :],
                    func=AF.Identity,
                    bias=nmr[:, i : i + 1],
                    scale=rstd[:, i : i + 1],
                )
                if affine:
                    mul_eng = nc.vector if i % 3 == 2 else nc.gpsimd
                    mul_eng.tensor_mul(
                        out_sb[:, slot, :], out_sb[:, slot, :], lnw_sb
                    )
                    nc.vector.tensor_add(
                        out_sb[:, slot, :], out_sb[:, slot, :], lnb_sb
                    )
                if i % 2 == 1:
                    # per-2-slot writeback so the DMA queue drains behind the
                    # LN chain instead of all at once after it
                    nc.sync.dma_start(
                        out=odst[:, i - 1 : i + 1, :],
                        in_=out_sb[:, slot - 1 : slot + 1, :],
                    )

        # ---- interleaved schedule ----
        # groups ordered so both batches' half-0 rows finish first: the
        # half-0 AR is kicked off mid-kernel and hides completely under the
        # half-1 groups; only the half-1 AR's latency is exposed, with the
        # half-0 LN applied underneath it. The next group's input DMAs are
        # queued before this group's chunk work (xpool bufs=4) so the serial
        # DMA queue stays ahead of PE.
        seq = [(0, 0), (1, 0), (0, 1), (1, 1)]
        tiles = (xtq0, xtk0)
        sqs = {}
        for i, (nl, half) in enumerate(seq):
            proj_mm(nl, half, *tiles)
            if i + 1 < len(seq):
                nnl, nhalf = seq[i + 1]
                tiles = (proj_dma(xqT, nnl, nhalf), proj_dma(xkT, nnl, nhalf))
            for ch in range(8 * half, 8 * half + 8):
                for hp in range(NHP):
                    prep(nl, hp, ch)
            for ch in range(8 * half, 8 * half + 8):
                for hp in range(NHP):
                    attn(nl, hp, ch)
                if ch == 8 * half + 5:
                    sqs[(nl, half, 0)] = stats_sq(nl, half, 0, 6)
            sqs[(nl, half, 6)] = stats_sq(nl, half, 6, 2)
            # DVE reduce/add finish passes deferred to just before the
            # gather that consumes them: emitted between groups they sit in
            # the in-order DVE queue ahead of the next group's attention
            # blends, stalling the chain and delaying the final collective
            if i in (1, 3):
                for pnl in range(NB):
                    stats_fin(pnl, half, 0, 6, sqs.pop((pnl, half, 0)))
                    stats_fin(pnl, half, 6, 2, sqs.pop((pnl, half, 6)))
                half_ar(half)
        # g-loads hoisted ahead of each half's LN bodies so the two batches'
        # LN chains pipeline across ACT/Pool/DVE instead of serializing
        # behind each other's output DMAs
        g00, g10 = ln_load(0, 0), ln_load(1, 0)
        ln_final(0, 0, g00)
        ln_final(1, 0, g10)
        g01, g11 = ln_load(0, 1), ln_load(1, 1)
        ln_final(0, 1, g01)
        ln_final(1, 1, g11)


_NC_CACHE = {}


def _get_nc(affine: bool = True):
    if affine not in _NC_CACHE:
        _install_patch()
        _NC_CACHE[affine] = build_nc(affine)
    return _NC_CACHE[affine]


def kernel(**inputs) -> np.ndarray:
    query = np.asarray(inputs["query"], np.float32)  # (L, N, E)
    key = np.asarray(inputs["key"], np.float32)
    Wq = np.asarray(inputs["Wq"], np.float32)        # (E, E)
    bq = np.asarray(inputs["bq"], np.float32)
    Wk = np.asarray(inputs["Wk"], np.float32)
    bk = np.asarray(inputs["bk"], np.float32)
    memory = np.asarray(inputs["memory"], np.float32)  # (MAXL, E)
    ln_w = np.asarray(inputs["ln_w"], np.float32)
    ln_b = np.asarray(inputs["ln_b"], np.float32)

    bf = ml_dtypes.bfloat16
    # (E, N, L) once; per group slice its two batches -> [E, 2L] batch-major
    xqT_all = np.ascontiguousarray(query.transpose(2, 1, 0))
    xkT_all = np.ascontiguousarray(key.transpose(2, 1, 0))
    xqT_g = [
        np.ascontiguousarray(xqT_all[:, 2 * g : 2 * g + 2, :].reshape(E, ROWSC)).astype(bf)
        for g in range(2)
    ]
    xkT_g = [
        np.ascontiguousarray(xkT_all[:, 2 * g : 2 * g + 2, :].reshape(E, ROWSC)).astype(bf)
        for g in range(2)
    ]

    # trivial LN affine (w==1, b==0) is compiled out of the kernel
    affine = not (np.all(ln_w == 1.0) and np.all(ln_b == 0.0))
    nc = _get_nc(affine)
    in_maps = []
    for c in range(NCORES):
        g, p = divmod(c, NGRP)
        sl = slice(p * ESL, (p + 1) * ESL)
        in_maps.append(
            {
                "xqT": xqT_g[g],
                "xkT": xkT_g[g],
                "wqT": np.ascontiguousarray(Wq[sl, :].T).astype(bf),
                "wkT": np.ascontiguousarray(Wk[sl, :].T).astype(bf),
                "bq": np.ascontiguousarray(bq[sl]),
                "bk": np.ascontiguousarray(bk[sl]),
                "memsc": (LAM * memory[:L, sl]).astype(bf),
                "lnw": np.ascontiguousarray(ln_w[sl]),
                "lnb": np.ascontiguousarray(ln_b[sl]),
            }
        )

    res = run_bass_kernel_spmd(nc, in_maps, core_ids=list(range(NCORES)))
    full = np.empty((L, N, E), np.float32)
    for c in range(NCORES):
        g, p = divmod(c, NGRP)
        o = res.results[c]["out"]  # (L, NB, ESL)
        for nl in range(NB):
            full[:, 2 * g + nl, p * ESL : (p + 1) * ESL] = o[:, nl, :]
    return full



# revision 51
# speedup vs baseline: 1.1478x; 1.1478x over previous
"""MemAttention Trainium2 kernel (8 NeuronCores, SPMD).

Math (see reference):
  q = gelu(query @ Wq.T + bq); k = gelu(key @ Wk.T + bk)        (erf gelu)
  mem = lam*memory + (1-lam)*q                                  (L == S == MAXL here)
  per (batch n, head h):  out = tril(qh @ kh.T) @ memh          (no softmax)
  out = LayerNorm_E(out) * ln_w + ln_b

Sharding: 2-way data-parallel over batch x 4-way tensor-parallel over heads.
Core c (group g = c//4, pos p = c%4) owns batches {2g, 2g+1} and heads
[4p, 4p+4) == E-slice [256p, 256p+256). Each core reads only its two batches'
(host-pre-transposed, bf16) query/key — half the input bytes of pure head
sharding — projects onto its 256-wide weight slice producing qT/kT in
[head*d, token] layout (two 128-partition head-pair blocks), and runs
attention for 2 batches x 2 head-pairs.

Attention uses the chunked linear-attention form (exact reassociation of the
causal masked product):
  A_i = sum_{s < i*C} k[s] (x) mem[s]          (d x d running state per head)
  out[chunk i] = tril(q_i k_i^T) @ mem_i + q_i @ A_i
  A_{i+1} = A_i + k_i^T @ mem_i
which needs O(L*C) score work instead of O(L^2).

LayerNorm is over the full E=1024, sharded 4-ways within each group: each
core contributes per-row (sum x, sum x^2)/1024 of its 256 columns; a 4-rank
AllGather per row-half (replica groups {0..3} and {4..7}) + a local 4-way
sum yields global stats (AllGather costs ~16.6us vs AllReduce's ~28.9us);
each core then normalizes and writes its (L, 2, 256) output block; the host
assembles the 4x4 blocks.

Scheduling notes (engines execute their static order in-order, so emission
order is the schedule):
 - projection row-tiles and the attention chunks they feed are interleaved so
   PE/DVE/ACT work hides under the input DMA stream;
 - per-chunk work is split into a state-independent "prep" (transposes, mem
   blend, masked diagonal scores — fully pipelined) and a short state-carrying
   "attn" part, with two head-pair streams interleaved to hide chain latency;
 - small collectives are latency-dominated and serialize, so stats use one
   AllGather + local sum per row-half (both batches): the half-0 gather
   hides fully under the half-1 groups, the half-1 gather is the only
   exposed collective latency, with half-0's LN applied underneath it;
 - stats are computed per 4-chunk quarter so the final AR isn't gated on a
   long post-attention reduction chain; LN g-loads are hoisted so the two
   quarters' LN chains pipeline across ACT/GPSIMD/DVE.
"""

import numpy as np
import ml_dtypes

import concourse.bass as bass
import concourse.mybir as mybir
import concourse.tile as tile
from concourse.bass_utils import run_bass_kernel_spmd
from concourse.masks import make_identity, make_upper_triangular

# ---------------------------------------------------------------------------
# Workaround: the walrus build in this container accepts only one sync-wait
# per instruction, but the Tile scheduler emits multi-wait Drains. Hoist the
# extra waits onto inserted NoOps (same engine, so execution order preserves
# semantics). Patched into both the native and the axon/PJRT compile paths.
# ---------------------------------------------------------------------------
import orjson

_MAX_WAITS = 1
_patch_done = False


def _split_waits(bir_json: bytes) -> bytes:
    d = orjson.loads(bir_json)
    n = 0
    for f in d.get("functions", []):
        for bb in f.get("blocks", []):
            instructions = bb.get("instructions")
            if not instructions:
                continue
            out = []
            changed = False
            for ins in instructions:
                si = ins.get("sync_info")
                waits = (si or {}).get("on_wait") or []
                if len(waits) > _MAX_WAITS:
                    changed = True
                    extra, keep = waits[:-_MAX_WAITS], waits[-_MAX_WAITS:]
                    for w in extra:
                        n += 1
                        out.append(
                            {
                                "debug": ins.get("debug", 0),
                                "engine": ins["engine"],
                                "ins": [],
                                "name": f"{ins.get('name', 'I')}-ws{n}",
                                "opcode": "NoOp",
                                "outs": [],
                                "sync_info": {"on_update": [], "on_wait": [w]},
                            }
                        )
                    si["on_wait"] = keep
                out.append(ins)
            if changed:
                bb["instructions"] = out
    return orjson.dumps(d)


def _install_patch():
    global _patch_done
    if _patch_done:
        return
    _patch_done = True
    import concourse.bass_utils as bass_utils
    import concourse.bass2jax as bass2jax

    orig = bass_utils.compile_bir_kernel

    def patched(bir_json, tmpdir, neff_name="file.neff"):
        return orig(_split_waits(bir_json), tmpdir, neff_name)

    bass_utils.compile_bir_kernel = patched
    bass2jax.compile_bir_kernel = patched


# ---------------------------------------------------------------------------
# Problem constants (hardcoded per contest contract)
# ---------------------------------------------------------------------------
L = 2048          # query length (== S == MAXL)
N = 4             # batch
E = 1024          # embed dim
H = 16            # heads
D = E // H        # head dim, 64
LAM = 0.001
LN_EPS = 1e-5
NCORES = 8
NGRP = 4           # cores per replica group (head-parallel within group)
NB = 2             # batches per core
NHP = 2            # head-pairs per core (4 heads)
ESL = 256          # per-core E slice
C = 128            # attention chunk
NCH = L // C       # 16 chunks per sequence
ROWSC = NB * L     # 4096 token rows per core, batch-major
KO = E // 128      # 8 contraction chunks
RT = 1024          # projection row-tile (L // 2)

F32 = mybir.dt.float32
BF16 = mybir.dt.bfloat16
AF = mybir.ActivationFunctionType
ALU = mybir.AluOpType


def _bc(ap, count, axis_pos=1):
    """Broadcast an AP by inserting a 0-stride dim of `count` at axis_pos."""
    new = list(ap.ap)
    new.insert(axis_pos, [0, count])
    return bass.AP(tensor=ap.tensor, offset=ap.offset, ap=new)


def build_nc(affine: bool = True) -> bass.Bass:
    nc = bass.Bass()

    # ---- I/O (per core: 2 batches, 256-wide head slice) ----
    xqT = nc.declare_dram_parameter("xqT", [E, ROWSC], BF16, isOutput=False)
    xkT = nc.declare_dram_parameter("xkT", [E, ROWSC], BF16, isOutput=False)
    wqT = nc.declare_dram_parameter("wqT", [E, ESL], BF16, isOutput=False)
    wkT = nc.declare_dram_parameter("wkT", [E, ESL], BF16, isOutput=False)
    bq = nc.declare_dram_parameter("bq", [ESL], F32, isOutput=False)
    bk = nc.declare_dram_parameter("bk", [ESL], F32, isOutput=False)
    memsc = nc.declare_dram_parameter("memsc", [L, ESL], BF16, isOutput=False)
    lnw = nc.declare_dram_parameter("lnw", [ESL], F32, isOutput=False)
    lnb = nc.declare_dram_parameter("lnb", [ESL], F32, isOutput=False)
    out = nc.declare_dram_parameter("out", [L, NB, ESL], F32, isOutput=True)

    # one stats buffer per ROW-HALF (both batches) -> one AllReduce per half;
    # with groups ordered (0,0),(1,0),(0,1),(1,1) the half-0 AR hides under
    # the half-1 groups and only the half-1 AR's latency is ever exposed
    cc_in = [nc.dram_tensor(f"cc_in{i}", [128, NB * (NCH // 2), 2], F32) for i in range(2)]
    # AllGather output: 4 rank blocks of [128, NCH, 2]; summed locally —
    # the cost model charges AllReduce 1.875x but AllGather only
    # (15us + bytes/40GBps), so gather+local-add halves the collective
    cc_out = [
        nc.dram_tensor(f"cc_out{i}", [NGRP * 128, NB * (NCH // 2), 2], F32)
        for i in range(2)
    ]

    with tile.TileContext(nc) as tc:
        _emit(nc, tc, xqT, xkT, wqT, wkT, bq, bk, memsc, lnw, lnb, out, cc_in, cc_out, affine)
    return nc


def _emit(nc, tc, xqT, xkT, wqT, wkT, bq, bk, memsc, lnw, lnb, out, cc_in, cc_out, affine):
    import contextlib

    ctx = contextlib.ExitStack()
    with ctx:
        singles = ctx.enter_context(tc.tile_pool(name="singles", bufs=1))
        # pools up front (no instructions emitted) so input DMAs can be
        # interleaved with the constant loads in queue order below
        xpool = ctx.enter_context(tc.tile_pool(name="xpool", bufs=4))
        ppool = ctx.enter_context(tc.tile_pool(name="ppool", bufs=2, space="PSUM"))
        tpool = ctx.enter_context(tc.tile_pool(name="tpool", bufs=2, space="PSUM"))
        spool = ctx.enter_context(tc.tile_pool(name="spool", bufs=2, space="PSUM"))
        opool = ctx.enter_context(tc.tile_pool(name="opool", bufs=2, space="PSUM"))
        apool = ctx.enter_context(tc.tile_pool(name="apool", bufs=3))
        stpool = ctx.enter_context(tc.tile_pool(name="stpool", bufs=2))

        def proj_dma(xdram, nl, rh):
            """Queue one row-tile's input stream; split by COLUMN halves so
            the first 512-col matmul group starts after 1MB, not 2MB."""
            r0 = nl * L + rh * RT
            xt = xpool.tile([128, KO, RT], BF16, tag="xt", name="xt")
            xsrc = xdram.rearrange("(ko ki) r -> ki ko r", ki=128)[:, :, r0 : r0 + RT]
            nc.sync.dma_start(out=xt[:, :, 0 : RT // 2], in_=xsrc[:, :, 0 : RT // 2])
            nc.sync.dma_start(out=xt[:, :, RT // 2 :], in_=xsrc[:, :, RT // 2 :])
            return xt

        # ---- DMA queue order: wq -> first xq -> wk -> first xk, so the
        # first matmul group is ready after ~2.5MB of stream instead of ~5MB;
        # mem/lnw/lnb (needed only later) queue after the first inputs ----
        wq_sb = singles.tile([128, KO, ESL], BF16)
        nc.sync.dma_start(out=wq_sb, in_=wqT.rearrange("(ko ki) m -> ki ko m", ki=128))
        # biases are 448B but gate the gelu evacuation (which gates PSUM
        # reuse) — keep them ahead of the megabyte input stream
        bq_sb = singles.tile([128, NHP], F32)
        nc.sync.dma_start(out=bq_sb, in_=bq.rearrange("(hp p) -> p hp", p=128))
        bk_sb = singles.tile([128, NHP], F32)
        nc.sync.dma_start(out=bk_sb, in_=bk.rearrange("(hp p) -> p hp", p=128))
        xtq0 = proj_dma(xqT, 0, 0)
        wk_sb = singles.tile([128, KO, ESL], BF16)
        nc.sync.dma_start(out=wk_sb, in_=wkT.rearrange("(ko ki) m -> ki ko m", ki=128))
        xtk0 = proj_dma(xkT, 0, 0)

        # ---- other constants (loaded while the first projections stream) ----
        ident = singles.tile([128, 128], BF16)
        make_identity(nc, ident)
        cmask = singles.tile([128, 128], F32)  # 1 where s(part) <= l(free)
        make_upper_triangular(nc, cmask, val=1.0, diag=True)
        cmask2 = singles.tile([128, 2, 128], F32)  # duplicated per head
        nc.vector.tensor_copy(cmask2[:, 0, :], cmask)
        nc.vector.tensor_copy(cmask2[:, 1, :], cmask)

        # lam*memory in [s%C, (chunk, e)] layout; first needed by prep(0,*,0)
        mem_sb = singles.tile([128, NCH, ESL], BF16)
        nc.sync.dma_start(out=mem_sb, in_=memsc.rearrange("(c p) e -> p c e", p=128))

        lnw_sb = singles.tile([128, ESL], F32)
        nc.sync.dma_start(out=lnw_sb, in_=_bc(lnw[:], 128, 0))
        lnb_sb = singles.tile([128, ESL], F32)
        nc.sync.dma_start(out=lnb_sb, in_=_bc(lnb[:], 128, 0))
        eps_sb = singles.tile([128, 1], F32)
        nc.vector.memset(eps_sb, LN_EPS)

        # ---- persistent activations: qT/kT per (batch, head-pair, row-half) ----
        qT_t = [
            [[singles.tile([128, RT], BF16, name=f"qT{nl}{hp}{rh}") for rh in range(2)]
             for hp in range(NHP)]
            for nl in range(NB)
        ]
        kT_t = [
            [[singles.tile([128, RT], BF16, name=f"kT{nl}{hp}{rh}") for rh in range(2)]
             for hp in range(NHP)]
            for nl in range(NB)
        ]
        out_sb = singles.tile([128, NB * NCH, ESL], F32)  # [l%C, (nl,ch), e]
        acc_sums = singles.tile([128, NB * NCH, NHP], F32)  # per-chunk/hp sum
        # linear-attention state per (batch, head-pair): [(h,d), s, e_local]
        a32 = singles.tile([128, NB * NHP, D], F32)
        abf = singles.tile([128, NB * NHP, D], BF16)

        def proj_mm(nl, rh, xtq, xtk):
            for xt, w_sb, bias_sb, dst in (
                (xtq, wq_sb, bq_sb, qT_t[nl]),
                (xtk, wk_sb, bk_sb, kT_t[nl]),
            ):
                # st-major: both head-pairs consume column-half st before
                # touching half st+1, matching the column-split DMA arrival
                # order so PE never stalls (and never drops p-state) mid-tile
                for st in range(RT // 512):
                    for hp in range(NHP):
                        ms = slice(hp * 128, (hp + 1) * 128)
                        ps = ppool.tile([128, 512], F32, tag="ps", name="ps")
                        for ko in range(KO):
                            nc.tensor.matmul(
                                ps,
                                w_sb[:, ko, ms],
                                xt[:, ko, st * 512 : (st + 1) * 512],
                                start=(ko == 0),
                                stop=(ko == KO - 1),
                            )
                        nc.scalar.activation(
                            out=dst[hp][rh][:, st * 512 : (st + 1) * 512],
                            in_=ps,
                            func=AF.Gelu,
                            bias=bias_sb[:, hp : hp + 1],
                            scale=1.0,
                        )

        pr_mem = {}
        pr_knat = {}
        pr_st = {}

        def prep(nl, hp, ch):
            """State-independent per-chunk work: transposes, mem blend,
            diagonal scores + causal mask. Fully pipelined, no serial chain."""
            rh, c0 = divmod(ch, 8)
            c0 *= C
            qTs = qT_t[nl][hp][rh][:, c0 : c0 + C]
            kTs = kT_t[nl][hp][rh][:, c0 : c0 + C]
            ms = slice(hp * 128, (hp + 1) * 128)

            # both transposes into one PSUM bank (sequential full-array PE
            # ops; explicit dep guards the half-0 read vs the half-1 write)
            qk_ps = tpool.tile([128, 2, 128], BF16, tag="tp", name="qk_ps")
            nc.tensor.transpose(qk_ps[:, 0, :], qTs, ident)
            i_t1 = nc.tensor.transpose(qk_ps[:, 1, :], kTs, ident)

            mem_nat = apool.tile([128, 128], BF16, tag="mn", bufs=12, name="mem_nat")
            i_stt = nc.vector.scalar_tensor_tensor(
                out=mem_nat,
                in0=qk_ps[:, 0, :],
                scalar=1.0 - LAM,
                in1=mem_sb[:, ch, ms],
                op0=ALU.mult,
                op1=ALU.add,
            )
            tile.add_dep_helper(i_stt.ins, i_t1.ins, reason="qk_ps bank serialize")
            knat = apool.tile([128, 128], BF16, tag="kn", bufs=12, name="knat")
            nc.scalar.copy(knat, qk_ps[:, 1, :])

            # the two heads' transposed scores go to DIFFERENT banks of one
            # 2-bank tile (concurrent row-packed MMs must not share a bank),
            # enabling a single merged causal-mask op
            st2_ps = spool.tile([128, 2, 512], F32, tag="st", bufs=1, name="st2_ps")
            for h in range(2):
                hs = slice(h * D, (h + 1) * D)
                nc.tensor.matmul(
                    st2_ps[:, h, 0:128], kTs[hs, :], qTs[hs, :], start=True, stop=True
                )
            st_sb = apool.tile([128, 2, 128], BF16, tag="stsb", bufs=12, name="st_sb")
            nc.vector.scalar_tensor_tensor(
                out=st_sb,
                in0=st2_ps[:, :, 0:128],
                scalar=1.0,
                in1=cmask2[:, :, :],
                op0=ALU.mult,
                op1=ALU.mult,
            )
            st_sbufs = [st_sb[:, 0, :], st_sb[:, 1, :]]
            pr_mem[(nl, hp, ch)] = mem_nat
            pr_knat[(nl, hp, ch)] = knat
            pr_st[(nl, hp, ch)] = st_sbufs

        def attn(nl, hp, ch):
            """State-carrying part: out = tril(S)@mem + q@A ; A += k^T@mem."""
            rh, c0 = divmod(ch, 8)
            c0 *= C
            slot = nl * NCH + ch
            sid = nl * NHP + hp
            qTs = qT_t[nl][hp][rh][:, c0 : c0 + C]
            mem_nat = pr_mem.pop((nl, hp, ch))
            knat = pr_knat.pop((nl, hp, ch))
            st_sbufs = pr_st.pop((nl, hp, ch))

            op_ps = opool.tile([128, 192], F32, tag="op", name="op_ps")
            for h in range(2):
                hs = slice(h * D, (h + 1) * D)
                nc.tensor.matmul(
                    op_ps[:, h * D : (h + 1) * D],
                    st_sbufs[h],
                    mem_nat[:, hs],
                    start=True,
                    stop=(ch == 0),
                )
                if ch > 0:
                    nc.tensor.matmul(
                        op_ps[:, h * D : (h + 1) * D],
                        qTs[hs, :],
                        abf[hs, sid, :],
                        start=False,
                        stop=True,
                    )
            i_d = None
            for h in range(2):
                hs = slice(h * D, (h + 1) * D)
                i_d = nc.tensor.matmul(
                    op_ps[hs, 128:192],
                    knat[:, hs],
                    mem_nat[:, hs],
                    start=True,
                    stop=True,
                )

            i_ev = nc.scalar.activation(
                out=out_sb[:, slot, hp * 128 : (hp + 1) * 128],
                in_=op_ps[:, 0:128],
                func=AF.Copy,
                accum_out=acc_sums[:, slot, hp : hp + 1],
            )
            # same PSUM bank: don't read cols 0:128 while PE writes 128:192
            tile.add_dep_helper(i_ev.ins, i_d.ins, reason="op_ps bank serialize")
            da = op_ps[:, 128:192]
            if ch == 0:
                nc.vector.tensor_copy(abf[:, sid, :], da)
                nc.vector.tensor_copy(a32[:, sid, :], da)
            else:
                # abf = bf16(a32_old + dA) first (reads old a32), then update
                # the fp32 accumulator; keeps ACT out of the state chain
                nc.vector.scalar_tensor_tensor(
                    out=abf[:, sid, :],
                    in0=da,
                    scalar=1.0,
                    in1=a32[:, sid, :],
                    op0=ALU.mult,
                    op1=ALU.add,
                )
                nc.vector.scalar_tensor_tensor(
                    out=a32[:, sid, :],
                    in0=da,
                    scalar=1.0,
                    in1=a32[:, sid, :],
                    op0=ALU.mult,
                    op1=ALU.add,
                )

        HCH = NCH // 2

        def stats_sq(nl, hf, lo, n):
            # x^2 on Pool only — Pool is not in the attention dependency
            # chain, so the 6-slot piece can run mid-attention without
            # stalling the DVE blends
            s0 = nl * NCH + hf * HCH + lo
            slab = out_sb[:, s0 : s0 + n, :]
            sq = stpool.tile([128, n, ESL], F32, tag=f"sq{n}", name="sq")
            nc.gpsimd.tensor_mul(sq, slab, slab)
            return sq

        def stats_fin(nl, hf, lo, n, sq):
            # mean/4 = sum(x)/1024 (evac accums), E[x^2]/4 = sum(x^2)/1024
            s0 = nl * NCH + hf * HCH + lo
            stats = stpool.tile([128, n, 2], F32, tag=f"stats{n}", name="stats")
            nc.vector.tensor_reduce(
                stats[:, :, 1], sq, axis=mybir.AxisListType.X, op=ALU.add
            )
            acc = acc_sums[:, s0 : s0 + n, :]
            nc.vector.tensor_add(stats[:, :, 0], acc[:, :, 0], acc[:, :, 1])
            nc.scalar.mul(stats, stats, 1.0 / E)
            nc.sync.dma_start(
                out=cc_in[hf][:, nl * HCH + lo : nl * HCH + lo + n, :],
                in_=stats,
            )

        def half_ar(hf):
            # single collective for this row-half of both batches; AllGather
            # + local sum instead of AllReduce: small collectives are
            # latency-dominated and AllReduce costs 1.875x the gather
            nc.gpsimd.collective_compute(
                "AllGather",
                ALU.bypass,
                replica_groups=[[0, 1, 2, 3], [4, 5, 6, 7]],
                ins=[cc_in[hf][:, :, :]],
                outs=[cc_out[hf][:, :, :]],
            )

        def ln_load(nl, hf):
            # fetch all 4 rank blocks and sum locally (order-invariant, so
            # the gather's rank-block layout doesn't matter)
            g4 = stpool.tile([128, NGRP, HCH, 2], F32, tag="g4", name="g4")
            nc.sync.dma_start(
                out=g4,
                in_=cc_out[hf].rearrange("(r p) c s -> p r c s", p=128)[
                    :, :, nl * HCH : (nl + 1) * HCH, :
                ],
            )
            g = stpool.tile([128, HCH, 2], F32, tag="g", name="g")
            nc.vector.tensor_add(g, g4[:, 0], g4[:, 1])
            nc.vector.tensor_add(g, g, g4[:, 2])
            nc.vector.tensor_add(g, g, g4[:, 3])
            return g

        def ln_final(nl, hf, g):
            s0 = nl * NCH + hf * HCH
            mu = g[:, :, 0]
            musq = stpool.tile([128, HCH], F32, tag="musq", name="musq")
            nc.vector.tensor_mul(musq, mu, mu)
            var = stpool.tile([128, HCH], F32, tag="var", name="var")
            nc.vector.tensor_sub(var, g[:, :, 1], musq)
            rstd = stpool.tile([128, HCH], F32, tag="rstd", name="rstd")
            nc.scalar.activation(out=rstd, in_=var, func=AF.Sqrt, bias=eps_sb, scale=1.0)
            nc.vector.reciprocal(rstd, rstd)
            nmr = stpool.tile([128, HCH], F32, tag="nmr", name="nmr")
            nc.vector.scalar_tensor_tensor(
                out=nmr, in0=mu, scalar=-1.0, op0=ALU.mult, in1=rstd, op1=ALU.mult
            )
            odst = out[hf * L // 2 : (hf + 1) * L // 2, nl, :].rearrange(
                "(c p_) e -> p_ c e", p_=128
            )
            for i in range(HCH):
                slot = s0 + i
                # (x - mu) * rstd on ACT, then * ln_w + ln_b; the muls are
                # split between Pool and DVE (Pool's ~600ns/op would other-
                # wise serialize the tail), adds stay on DVE
                nc.scalar.activation(
                    out=out_sb[:, slot, :],
                    in_=out_sb[:, slot, :],
                    func=AF.Identity,
                    bias=nmr[:, i : i + 1],
                    scale=rstd[:, i : i + 1],
                )
                if affine:
                    mul_eng = nc.vector if i % 3 == 2 else nc.gpsimd
                    mul_eng.tensor_mul(
                        out_sb[:, slot, :], out_sb[:, slot, :], lnw_sb
                    )
                    nc.vector.tensor_add(
                        out_sb[:, slot, :], out_sb[:, slot, :], lnb_sb
                    )
                if i % 2 == 1:
                    # per-2-slot writeback so the DMA queue drains behind the
                    # LN chain instead of all at once after it
                    nc.sync.dma_start(
                        out=odst[:, i - 1 : i + 1, :],
                        in_=out_sb[:, slot - 1 : slot + 1, :],
                    )

        # ---- interleaved schedule ----
        # groups ordered so both batches' half-0 rows finish first: the
        # half-0 AR is kicked off mid-kernel and hides completely under the
        # half-1 groups; only the half-1 AR's latency is exposed, with the
        # half-0 LN applied underneath it. The next group's input DMAs are
        # queued before this group's chunk work (xpool bufs=4) so the serial
        # DMA queue stays ahead of PE.
        seq = [(0, 0), (1, 0), (0, 1), (1, 1)]
        tiles = (xtq0, xtk0)
        sqs = {}
        for i, (nl, half) in enumerate(seq):
            proj_mm(nl, half, *tiles)
            if i + 1 < len(seq):
                nnl, nhalf = seq[i + 1]
                tiles = (proj_dma(xqT, nnl, nhalf), proj_dma(xkT, nnl, nhalf))
            for ch in range(8 * half, 8 * half + 8):
                for hp in range(NHP):
                    prep(nl, hp, ch)
            for ch in range(8 * half, 8 * half + 8):
                for hp in range(NHP):
                    attn(nl, hp, ch)
                if ch == 8 * half + 5:
                    sqs[(nl, half, 0)] = stats_sq(nl, half, 0, 6)
            sqs[(nl, half, 6)] = stats_sq(nl, half, 6, 2)
            # DVE reduce/add finish passes deferred to just before the
            # gather that consumes them: emitted between groups they sit in
            # the in-order DVE queue ahead of the next group's attention
            # blends, stalling the chain and delaying the final collective
            if i in (1, 3):
                for pnl in range(NB):
                    stats_fin(pnl, half, 0, 6, sqs.pop((pnl, half, 0)))
                    stats_fin(pnl, half, 6, 2, sqs.pop((pnl, half, 6)))
                half_ar(half)
        # g-loads hoisted ahead of each half's LN bodies so the two batches'
        # LN chains pipeline across ACT/Pool/DVE instead of serializing
        # behind each other's output DMAs
        g00, g10 = ln_load(0, 0), ln_load(1, 0)
        ln_final(0, 0, g00)
        ln_final(1, 0, g10)
        g01, g11 = ln_load(0, 1), ln_load(1, 1)
        ln_final(0, 1, g01)
        ln_final(1, 1, g11)


_NC_CACHE = {}


def _get_nc(affine: bool = True):
    if affine not in _NC_CACHE:
        _install_patch()
        _NC_CACHE[affine] = build_nc(affine)
    return _NC_CACHE[affine]


def kernel(**inputs) -> np.ndarray:
    query = np.asarray(inputs["query"], np.float32)  # (L, N, E)
    key = np.asarray(inputs["key"], np.float32)
    Wq = np.asarray(inputs["Wq"], np.float32)        # (E, E)
    bq = np.asarray(inputs["bq"], np.float32)
    Wk = np.asarray(inputs["Wk"], np.float32)
    bk = np.asarray(inputs["bk"], np.float32)
    memory = np.asarray(inputs["memory"], np.float32)  # (MAXL, E)
    ln_w = np.asarray(inputs["ln_w"], np.float32)
    ln_b = np.asarray(inputs["ln_b"], np.float32)

    bf = ml_dtypes.bfloat16
    # (E, N, L) once; per group slice its two batches -> [E, 2L] batch-major
    xqT_all = np.ascontiguousarray(query.transpose(2, 1, 0))
    xkT_all = np.ascontiguousarray(key.transpose(2, 1, 0))
    xqT_g = [
        np.ascontiguousarray(xqT_all[:, 2 * g : 2 * g + 2, :].reshape(E, ROWSC)).astype(bf)
        for g in range(2)
    ]
    xkT_g = [
        np.ascontiguousarray(xkT_all[:, 2 * g : 2 * g + 2, :].reshape(E, ROWSC)).astype(bf)
        for g in range(2)
    ]

    # trivial LN affine (w==1, b==0) is compiled out of the kernel
    affine = not (np.all(ln_w == 1.0) and np.all(ln_b == 0.0))
    nc = _get_nc(affine)
    in_maps = []
    for c in range(NCORES):
        g, p = divmod(c, NGRP)
        sl = slice(p * ESL, (p + 1) * ESL)
        in_maps.append(
            {
                "xqT": xqT_g[g],
                "xkT": xkT_g[g],
                "wqT": np.ascontiguousarray(Wq[sl, :].T).astype(bf),
                "wkT": np.ascontiguousarray(Wk[sl, :].T).astype(bf),
                "bq": np.ascontiguousarray(bq[sl]),
                "bk": np.ascontiguousarray(bk[sl]),
                "memsc": (LAM * memory[:L, sl]).astype(bf),
                "lnw": np.ascontiguousarray(ln_w[sl]),
                "lnb": np.ascontiguousarray(ln_b[sl]),
            }
        )

    res = run_bass_kernel_spmd(nc, in_maps, core_ids=list(range(NCORES)))
    full = np.empty((L, N, E), np.float32)
    for c in range(NCORES):
        g, p = divmod(c, NGRP)
        o = res.results[c]["out"]  # (L, NB, ESL)
        for nl in range(NB):
            full[:, 2 * g + nl, p * ESL : (p + 1) * ESL] = o[:, nl, :]
    return full



# revision 52
# speedup vs baseline: 1.1479x; 1.0001x over previous
"""MemAttention Trainium2 kernel (8 NeuronCores, SPMD).

Math (see reference):
  q = gelu(query @ Wq.T + bq); k = gelu(key @ Wk.T + bk)        (erf gelu)
  mem = lam*memory + (1-lam)*q                                  (L == S == MAXL here)
  per (batch n, head h):  out = tril(qh @ kh.T) @ memh          (no softmax)
  out = LayerNorm_E(out) * ln_w + ln_b

Sharding: 2-way data-parallel over batch x 4-way tensor-parallel over heads.
Core c (group g = c//4, pos p = c%4) owns batches {2g, 2g+1} and heads
[4p, 4p+4) == E-slice [256p, 256p+256). Each core reads only its two batches'
(host-pre-transposed, bf16) query/key — half the input bytes of pure head
sharding — projects onto its 256-wide weight slice producing qT/kT in
[head*d, token] layout (two 128-partition head-pair blocks), and runs
attention for 2 batches x 2 head-pairs.

Attention uses the chunked linear-attention form (exact reassociation of the
causal masked product):
  A_i = sum_{s < i*C} k[s] (x) mem[s]          (d x d running state per head)
  out[chunk i] = tril(q_i k_i^T) @ mem_i + q_i @ A_i
  A_{i+1} = A_i + k_i^T @ mem_i
which needs O(L*C) score work instead of O(L^2).

LayerNorm is over the full E=1024, sharded 4-ways within each group: each
core contributes per-row (sum x, sum x^2)/1024 of its 256 columns; a 4-rank
AllGather per row-half (replica groups {0..3} and {4..7}) + a local 4-way
sum yields global stats (AllGather costs ~16.6us vs AllReduce's ~28.9us);
each core then normalizes and writes its (L, 2, 256) output block; the host
assembles the 4x4 blocks.

Scheduling notes (engines execute their static order in-order, so emission
order is the schedule):
 - projection row-tiles and the attention chunks they feed are interleaved so
   PE/DVE/ACT work hides under the input DMA stream;
 - per-chunk work is split into a state-independent "prep" (transposes, mem
   blend, masked diagonal scores — fully pipelined) and a short state-carrying
   "attn" part, with two head-pair streams interleaved to hide chain latency;
 - small collectives are latency-dominated and serialize, so stats use one
   AllGather + local sum per row-half (both batches): the half-0 gather
   hides fully under the half-1 groups, the half-1 gather is the only
   exposed collective latency, with half-0's LN applied underneath it;
 - stats are computed per 4-chunk quarter so the final AR isn't gated on a
   long post-attention reduction chain; LN g-loads are hoisted so the two
   quarters' LN chains pipeline across ACT/GPSIMD/DVE.
"""

import numpy as np
import ml_dtypes

import concourse.bass as bass
import concourse.mybir as mybir
import concourse.tile as tile
from concourse.bass_utils import run_bass_kernel_spmd
from concourse.masks import make_identity, make_upper_triangular

# ---------------------------------------------------------------------------
# Workaround: the walrus build in this container accepts only one sync-wait
# per instruction, but the Tile scheduler emits multi-wait Drains. Hoist the
# extra waits onto inserted NoOps (same engine, so execution order preserves
# semantics). Patched into both the native and the axon/PJRT compile paths.
# ---------------------------------------------------------------------------
import orjson

_MAX_WAITS = 1
_patch_done = False


def _split_waits(bir_json: bytes) -> bytes:
    d = orjson.loads(bir_json)
    n = 0
    for f in d.get("functions", []):
        for bb in f.get("blocks", []):
            instructions = bb.get("instructions")
            if not instructions:
                continue
            out = []
            changed = False
            for ins in instructions:
                si = ins.get("sync_info")
                waits = (si or {}).get("on_wait") or []
                if len(waits) > _MAX_WAITS:
                    changed = True
                    extra, keep = waits[:-_MAX_WAITS], waits[-_MAX_WAITS:]
                    for w in extra:
                        n += 1
                        out.append(
                            {
                                "debug": ins.get("debug", 0),
                                "engine": ins["engine"],
                                "ins": [],
                                "name": f"{ins.get('name', 'I')}-ws{n}",
                                "opcode": "NoOp",
                                "outs": [],
                                "sync_info": {"on_update": [], "on_wait": [w]},
                            }
                        )
                    si["on_wait"] = keep
                out.append(ins)
            if changed:
                bb["instructions"] = out
    return orjson.dumps(d)


def _install_patch():
    global _patch_done
    if _patch_done:
        return
    _patch_done = True
    import concourse.bass_utils as bass_utils
    import concourse.bass2jax as bass2jax

    orig = bass_utils.compile_bir_kernel

    def patched(bir_json, tmpdir, neff_name="file.neff"):
        return orig(_split_waits(bir_json), tmpdir, neff_name)

    bass_utils.compile_bir_kernel = patched
    bass2jax.compile_bir_kernel = patched


# ---------------------------------------------------------------------------
# Problem constants (hardcoded per contest contract)
# ---------------------------------------------------------------------------
L = 2048          # query length (== S == MAXL)
N = 4             # batch
E = 1024          # embed dim
H = 16            # heads
D = E // H        # head dim, 64
LAM = 0.001
LN_EPS = 1e-5
NCORES = 8
NGRP = 4           # cores per replica group (head-parallel within group)
NB = 2             # batches per core
NHP = 2            # head-pairs per core (4 heads)
ESL = 256          # per-core E slice
C = 128            # attention chunk
NCH = L // C       # 16 chunks per sequence
ROWSC = NB * L     # 4096 token rows per core, batch-major
KO = E // 128      # 8 contraction chunks
RT = 1024          # projection row-tile (L // 2)

F32 = mybir.dt.float32
BF16 = mybir.dt.bfloat16
AF = mybir.ActivationFunctionType
ALU = mybir.AluOpType


def _bc(ap, count, axis_pos=1):
    """Broadcast an AP by inserting a 0-stride dim of `count` at axis_pos."""
    new = list(ap.ap)
    new.insert(axis_pos, [0, count])
    return bass.AP(tensor=ap.tensor, offset=ap.offset, ap=new)


def build_nc(affine: bool = True) -> bass.Bass:
    nc = bass.Bass()

    # ---- I/O (per core: 2 batches, 256-wide head slice) ----
    xqT = nc.declare_dram_parameter("xqT", [E, ROWSC], BF16, isOutput=False)
    xkT = nc.declare_dram_parameter("xkT", [E, ROWSC], BF16, isOutput=False)
    wqT = nc.declare_dram_parameter("wqT", [E, ESL], BF16, isOutput=False)
    wkT = nc.declare_dram_parameter("wkT", [E, ESL], BF16, isOutput=False)
    bq = nc.declare_dram_parameter("bq", [ESL], F32, isOutput=False)
    bk = nc.declare_dram_parameter("bk", [ESL], F32, isOutput=False)
    memsc = nc.declare_dram_parameter("memsc", [L, ESL], BF16, isOutput=False)
    lnw = nc.declare_dram_parameter("lnw", [ESL], F32, isOutput=False)
    lnb = nc.declare_dram_parameter("lnb", [ESL], F32, isOutput=False)
    out = nc.declare_dram_parameter("out", [L, NB, ESL], F32, isOutput=True)

    # one stats buffer per ROW-HALF (both batches) -> one AllReduce per half;
    # with groups ordered (0,0),(1,0),(0,1),(1,1) the half-0 AR hides under
    # the half-1 groups and only the half-1 AR's latency is ever exposed
    cc_in = [nc.dram_tensor(f"cc_in{i}", [128, NB * (NCH // 2), 2], F32) for i in range(2)]
    # AllGather output: 4 rank blocks of [128, NCH, 2]; summed locally —
    # the cost model charges AllReduce 1.875x but AllGather only
    # (15us + bytes/40GBps), so gather+local-add halves the collective
    cc_out = [
        nc.dram_tensor(f"cc_out{i}", [NGRP * 128, NB * (NCH // 2), 2], F32)
        for i in range(2)
    ]

    with tile.TileContext(nc) as tc:
        _emit(nc, tc, xqT, xkT, wqT, wkT, bq, bk, memsc, lnw, lnb, out, cc_in, cc_out, affine)
    return nc


def _emit(nc, tc, xqT, xkT, wqT, wkT, bq, bk, memsc, lnw, lnb, out, cc_in, cc_out, affine):
    import contextlib

    ctx = contextlib.ExitStack()
    with ctx:
        singles = ctx.enter_context(tc.tile_pool(name="singles", bufs=1))
        # pools up front (no instructions emitted) so input DMAs can be
        # interleaved with the constant loads in queue order below
        xpool = ctx.enter_context(tc.tile_pool(name="xpool", bufs=4))
        ppool = ctx.enter_context(tc.tile_pool(name="ppool", bufs=2, space="PSUM"))
        tpool = ctx.enter_context(tc.tile_pool(name="tpool", bufs=2, space="PSUM"))
        spool = ctx.enter_context(tc.tile_pool(name="spool", bufs=2, space="PSUM"))
        opool = ctx.enter_context(tc.tile_pool(name="opool", bufs=2, space="PSUM"))
        apool = ctx.enter_context(tc.tile_pool(name="apool", bufs=3))
        stpool = ctx.enter_context(tc.tile_pool(name="stpool", bufs=2))

        def proj_dma(xdram, nl, rh):
            """Queue one row-tile's input stream; split by COLUMN halves so
            the first 512-col matmul group starts after 1MB, not 2MB."""
            r0 = nl * L + rh * RT
            xt = xpool.tile([128, KO, RT], BF16, tag="xt", name="xt")
            xsrc = xdram.rearrange("(ko ki) r -> ki ko r", ki=128)[:, :, r0 : r0 + RT]
            nc.sync.dma_start(out=xt[:, :, 0 : RT // 2], in_=xsrc[:, :, 0 : RT // 2])
            nc.sync.dma_start(out=xt[:, :, RT // 2 :], in_=xsrc[:, :, RT // 2 :])
            return xt

        # ---- DMA queue order: wq -> first xq -> wk -> first xk, so the
        # first matmul group is ready after ~2.5MB of stream instead of ~5MB;
        # mem/lnw/lnb (needed only later) queue after the first inputs ----
        wq_sb = singles.tile([128, KO, ESL], BF16)
        nc.sync.dma_start(out=wq_sb, in_=wqT.rearrange("(ko ki) m -> ki ko m", ki=128))
        # biases are 448B but gate the gelu evacuation (which gates PSUM
        # reuse) — keep them ahead of the megabyte input stream
        bq_sb = singles.tile([128, NHP], F32)
        nc.sync.dma_start(out=bq_sb, in_=bq.rearrange("(hp p) -> p hp", p=128))
        bk_sb = singles.tile([128, NHP], F32)
        nc.sync.dma_start(out=bk_sb, in_=bk.rearrange("(hp p) -> p hp", p=128))
        xtq0 = proj_dma(xqT, 0, 0)
        wk_sb = singles.tile([128, KO, ESL], BF16)
        nc.sync.dma_start(out=wk_sb, in_=wkT.rearrange("(ko ki) m -> ki ko m", ki=128))
        xtk0 = proj_dma(xkT, 0, 0)

        # ---- other constants (loaded while the first projections stream) ----
        ident = singles.tile([128, 128], BF16)
        make_identity(nc, ident)
        cmask = singles.tile([128, 128], F32)  # 1 where s(part) <= l(free)
        make_upper_triangular(nc, cmask, val=1.0, diag=True)
        cmask2 = singles.tile([128, 2, 128], F32)  # duplicated per head
        nc.vector.tensor_copy(cmask2[:, 0, :], cmask)
        nc.vector.tensor_copy(cmask2[:, 1, :], cmask)

        # lam*memory in [s%C, (chunk, e)] layout; first needed by prep(0,*,0)
        mem_sb = singles.tile([128, NCH, ESL], BF16)
        nc.sync.dma_start(out=mem_sb, in_=memsc.rearrange("(c p) e -> p c e", p=128))

        lnw_sb = singles.tile([128, ESL], F32)
        nc.sync.dma_start(out=lnw_sb, in_=_bc(lnw[:], 128, 0))
        lnb_sb = singles.tile([128, ESL], F32)
        nc.sync.dma_start(out=lnb_sb, in_=_bc(lnb[:], 128, 0))
        eps_sb = singles.tile([128, 1], F32)
        nc.vector.memset(eps_sb, LN_EPS)

        # ---- persistent activations: qT/kT per (batch, head-pair, row-half) ----
        qT_t = [
            [[singles.tile([128, RT], BF16, name=f"qT{nl}{hp}{rh}") for rh in range(2)]
             for hp in range(NHP)]
            for nl in range(NB)
        ]
        kT_t = [
            [[singles.tile([128, RT], BF16, name=f"kT{nl}{hp}{rh}") for rh in range(2)]
             for hp in range(NHP)]
            for nl in range(NB)
        ]
        out_sb = singles.tile([128, NB * NCH, ESL], F32)  # [l%C, (nl,ch), e]
        acc_sums = singles.tile([128, NB * NCH, NHP], F32)  # per-chunk/hp sum
        # linear-attention state per (batch, head-pair): [(h,d), s, e_local]
        a32 = singles.tile([128, NB * NHP, D], F32)
        abf = singles.tile([128, NB * NHP, D], BF16)

        def proj_mm(nl, rh, xtq, xtk):
            for xt, w_sb, bias_sb, dst in (
                (xtq, wq_sb, bq_sb, qT_t[nl]),
                (xtk, wk_sb, bk_sb, kT_t[nl]),
            ):
                # st-major: both head-pairs consume column-half st before
                # touching half st+1, matching the column-split DMA arrival
                # order so PE never stalls (and never drops p-state) mid-tile
                for st in range(RT // 512):
                    for hp in range(NHP):
                        ms = slice(hp * 128, (hp + 1) * 128)
                        ps = ppool.tile([128, 512], F32, tag="ps", name="ps")
                        for ko in range(KO):
                            nc.tensor.matmul(
                                ps,
                                w_sb[:, ko, ms],
                                xt[:, ko, st * 512 : (st + 1) * 512],
                                start=(ko == 0),
                                stop=(ko == KO - 1),
                            )
                        nc.scalar.activation(
                            out=dst[hp][rh][:, st * 512 : (st + 1) * 512],
                            in_=ps,
                            func=AF.Gelu,
                            bias=bias_sb[:, hp : hp + 1],
                            scale=1.0,
                        )

        pr_mem = {}
        pr_knat = {}
        pr_st = {}

        def prep(nl, hp, ch):
            """State-independent per-chunk work: transposes, mem blend,
            diagonal scores + causal mask. Fully pipelined, no serial chain."""
            rh, c0 = divmod(ch, 8)
            c0 *= C
            qTs = qT_t[nl][hp][rh][:, c0 : c0 + C]
            kTs = kT_t[nl][hp][rh][:, c0 : c0 + C]
            ms = slice(hp * 128, (hp + 1) * 128)

            # both transposes into one PSUM bank (sequential full-array PE
            # ops; explicit dep guards the half-0 read vs the half-1 write)
            qk_ps = tpool.tile([128, 2, 128], BF16, tag="tp", name="qk_ps")
            nc.tensor.transpose(qk_ps[:, 0, :], qTs, ident)
            i_t1 = nc.tensor.transpose(qk_ps[:, 1, :], kTs, ident)

            mem_nat = apool.tile([128, 128], BF16, tag="mn", bufs=12, name="mem_nat")
            i_stt = nc.vector.scalar_tensor_tensor(
                out=mem_nat,
                in0=qk_ps[:, 0, :],
                scalar=1.0 - LAM,
                in1=mem_sb[:, ch, ms],
                op0=ALU.mult,
                op1=ALU.add,
            )
            tile.add_dep_helper(i_stt.ins, i_t1.ins, reason="qk_ps bank serialize")
            knat = apool.tile([128, 128], BF16, tag="kn", bufs=12, name="knat")
            nc.scalar.copy(knat, qk_ps[:, 1, :])

            # the two heads' transposed scores go to DIFFERENT banks of one
            # 2-bank tile (concurrent row-packed MMs must not share a bank),
            # enabling a single merged causal-mask op
            st2_ps = spool.tile([128, 2, 512], F32, tag="st", bufs=1, name="st2_ps")
            for h in range(2):
                hs = slice(h * D, (h + 1) * D)
                nc.tensor.matmul(
                    st2_ps[:, h, 0:128], kTs[hs, :], qTs[hs, :], start=True, stop=True
                )
            st_sb = apool.tile([128, 2, 128], BF16, tag="stsb", bufs=12, name="st_sb")
            nc.vector.scalar_tensor_tensor(
                out=st_sb,
                in0=st2_ps[:, :, 0:128],
                scalar=1.0,
                in1=cmask2[:, :, :],
                op0=ALU.mult,
                op1=ALU.mult,
            )
            st_sbufs = [st_sb[:, 0, :], st_sb[:, 1, :]]
            pr_mem[(nl, hp, ch)] = mem_nat
            pr_knat[(nl, hp, ch)] = knat
            pr_st[(nl, hp, ch)] = st_sbufs

        def attn(nl, hp, ch):
            """State-carrying part: out = tril(S)@mem + q@A ; A += k^T@mem."""
            rh, c0 = divmod(ch, 8)
            c0 *= C
            slot = nl * NCH + ch
            sid = nl * NHP + hp
            qTs = qT_t[nl][hp][rh][:, c0 : c0 + C]
            mem_nat = pr_mem.pop((nl, hp, ch))
            knat = pr_knat.pop((nl, hp, ch))
            st_sbufs = pr_st.pop((nl, hp, ch))

            op_ps = opool.tile([128, 192], F32, tag="op", name="op_ps")
            for h in range(2):
                hs = slice(h * D, (h + 1) * D)
                nc.tensor.matmul(
                    op_ps[:, h * D : (h + 1) * D],
                    st_sbufs[h],
                    mem_nat[:, hs],
                    start=True,
                    stop=(ch == 0),
                )
                if ch > 0:
                    nc.tensor.matmul(
                        op_ps[:, h * D : (h + 1) * D],
                        qTs[hs, :],
                        abf[hs, sid, :],
                        start=False,
                        stop=True,
                    )
            i_d = None
            for h in range(2):
                hs = slice(h * D, (h + 1) * D)
                i_d = nc.tensor.matmul(
                    op_ps[hs, 128:192],
                    knat[:, hs],
                    mem_nat[:, hs],
                    start=True,
                    stop=True,
                )

            i_ev = nc.scalar.activation(
                out=out_sb[:, slot, hp * 128 : (hp + 1) * 128],
                in_=op_ps[:, 0:128],
                func=AF.Copy,
                accum_out=acc_sums[:, slot, hp : hp + 1],
            )
            # same PSUM bank: don't read cols 0:128 while PE writes 128:192
            tile.add_dep_helper(i_ev.ins, i_d.ins, reason="op_ps bank serialize")
            da = op_ps[:, 128:192]
            if ch == 0:
                nc.vector.tensor_copy(abf[:, sid, :], da)
                nc.vector.tensor_copy(a32[:, sid, :], da)
            else:
                # abf = bf16(a32_old + dA) first (reads old a32), then update
                # the fp32 accumulator; keeps ACT out of the state chain
                nc.vector.scalar_tensor_tensor(
                    out=abf[:, sid, :],
                    in0=da,
                    scalar=1.0,
                    in1=a32[:, sid, :],
                    op0=ALU.mult,
                    op1=ALU.add,
                )
                nc.vector.scalar_tensor_tensor(
                    out=a32[:, sid, :],
                    in0=da,
                    scalar=1.0,
                    in1=a32[:, sid, :],
                    op0=ALU.mult,
                    op1=ALU.add,
                )

        HCH = NCH // 2

        def stats_sq(nl, hf, lo, n):
            # x^2 on Pool only — Pool is not in the attention dependency
            # chain, so the 6-slot piece can run mid-attention without
            # stalling the DVE blends
            s0 = nl * NCH + hf * HCH + lo
            slab = out_sb[:, s0 : s0 + n, :]
            sq = stpool.tile([128, n, ESL], F32, tag=f"sq{n}", name="sq")
            nc.gpsimd.tensor_mul(sq, slab, slab)
            return sq

        def stats_fin(nl, hf, lo, n, sq):
            # mean/4 = sum(x)/1024 (evac accums), E[x^2]/4 = sum(x^2)/1024
            s0 = nl * NCH + hf * HCH + lo
            stats = stpool.tile([128, n, 2], F32, tag=f"stats{n}", name="stats")
            nc.vector.tensor_reduce(
                stats[:, :, 1], sq, axis=mybir.AxisListType.X, op=ALU.add
            )
            acc = acc_sums[:, s0 : s0 + n, :]
            nc.vector.tensor_add(stats[:, :, 0], acc[:, :, 0], acc[:, :, 1])
            nc.scalar.mul(stats, stats, 1.0 / E)
            nc.sync.dma_start(
                out=cc_in[hf][:, nl * HCH + lo : nl * HCH + lo + n, :],
                in_=stats,
            )

        def half_ar(hf):
            # single collective for this row-half of both batches; AllGather
            # + local sum instead of AllReduce: small collectives are
            # latency-dominated and AllReduce costs 1.875x the gather
            nc.gpsimd.collective_compute(
                "AllGather",
                ALU.bypass,
                replica_groups=[[0, 1, 2, 3], [4, 5, 6, 7]],
                ins=[cc_in[hf][:, :, :]],
                outs=[cc_out[hf][:, :, :]],
            )

        def ln_load(nl, hf):
            # fetch all 4 rank blocks and sum locally (order-invariant, so
            # the gather's rank-block layout doesn't matter)
            g4 = stpool.tile([128, NGRP, HCH, 2], F32, tag="g4", name="g4")
            nc.sync.dma_start(
                out=g4,
                in_=cc_out[hf].rearrange("(r p) c s -> p r c s", p=128)[
                    :, :, nl * HCH : (nl + 1) * HCH, :
                ],
            )
            g = stpool.tile([128, HCH, 2], F32, tag="g", name="g")
            # one strided reduce over the rank dim instead of 3 serial adds:
            # DVE reads SBUF with arbitrary strides, so the DMA layout stays
            # rank-block contiguous and only the reduce view is permuted
            nc.vector.tensor_reduce(
                g,
                g4.rearrange("p r c s -> p c s r"),
                axis=mybir.AxisListType.X,
                op=ALU.add,
            )
            return g

        def ln_final(nl, hf, g):
            s0 = nl * NCH + hf * HCH
            mu = g[:, :, 0]
            musq = stpool.tile([128, HCH], F32, tag="musq", name="musq")
            nc.vector.tensor_mul(musq, mu, mu)
            var = stpool.tile([128, HCH], F32, tag="var", name="var")
            nc.vector.tensor_sub(var, g[:, :, 1], musq)
            rstd = stpool.tile([128, HCH], F32, tag="rstd", name="rstd")
            nc.scalar.activation(out=rstd, in_=var, func=AF.Sqrt, bias=eps_sb, scale=1.0)
            nc.vector.reciprocal(rstd, rstd)
            nmr = stpool.tile([128, HCH], F32, tag="nmr", name="nmr")
            nc.vector.scalar_tensor_tensor(
                out=nmr, in0=mu, scalar=-1.0, op0=ALU.mult, in1=rstd, op1=ALU.mult
            )
            odst = out[hf * L // 2 : (hf + 1) * L // 2, nl, :].rearrange(
                "(c p_) e -> p_ c e", p_=128
            )
            for i in range(HCH):
                slot = s0 + i
                # (x - mu) * rstd on ACT, then * ln_w + ln_b; the muls are
                # split between Pool and DVE (Pool's ~600ns/op would other-
                # wise serialize the tail), adds stay on DVE
                nc.scalar.activation(
                    out=out_sb[:, slot, :],
                    in_=out_sb[:, slot, :],
                    func=AF.Identity,
                    bias=nmr[:, i : i + 1],
                    scale=rstd[:, i : i + 1],
                )
                if affine:
                    mul_eng = nc.vector if i % 3 == 2 else nc.gpsimd
                    mul_eng.tensor_mul(
                        out_sb[:, slot, :], out_sb[:, slot, :], lnw_sb
                    )
                    nc.vector.tensor_add(
                        out_sb[:, slot, :], out_sb[:, slot, :], lnb_sb
                    )
                if i % 2 == 1:
                    # per-2-slot writeback so the DMA queue drains behind the
                    # LN chain instead of all at once after it
                    nc.sync.dma_start(
                        out=odst[:, i - 1 : i + 1, :],
                        in_=out_sb[:, slot - 1 : slot + 1, :],
                    )

        # ---- interleaved schedule ----
        # groups ordered so both batches' half-0 rows finish first: the
        # half-0 AR is kicked off mid-kernel and hides completely under the
        # half-1 groups; only the half-1 AR's latency is exposed, with the
        # half-0 LN applied underneath it. The next group's input DMAs are
        # queued before this group's chunk work (xpool bufs=4) so the serial
        # DMA queue stays ahead of PE.
        seq = [(0, 0), (1, 0), (0, 1), (1, 1)]
        tiles = (xtq0, xtk0)
        sqs = {}
        for i, (nl, half) in enumerate(seq):
            proj_mm(nl, half, *tiles)
            if i + 1 < len(seq):
                nnl, nhalf = seq[i + 1]
                tiles = (proj_dma(xqT, nnl, nhalf), proj_dma(xkT, nnl, nhalf))
            for ch in range(8 * half, 8 * half + 8):
                for hp in range(NHP):
                    prep(nl, hp, ch)
            for ch in range(8 * half, 8 * half + 8):
                for hp in range(NHP):
                    attn(nl, hp, ch)
                if ch == 8 * half + 5:
                    sqs[(nl, half, 0)] = stats_sq(nl, half, 0, 6)
            sqs[(nl, half, 6)] = stats_sq(nl, half, 6, 2)
            # DVE reduce/add finish passes deferred to just before the
            # gather that consumes them: emitted between groups they sit in
            # the in-order DVE queue ahead of the next group's attention
            # blends, stalling the chain and delaying the final collective
            if i in (1, 3):
                for pnl in range(NB):
                    stats_fin(pnl, half, 0, 6, sqs.pop((pnl, half, 0)))
                    stats_fin(pnl, half, 6, 2, sqs.pop((pnl, half, 6)))
                half_ar(half)
        # g-loads hoisted ahead of each half's LN bodies so the two batches'
        # LN chains pipeline across ACT/Pool/DVE instead of serializing
        # behind each other's output DMAs
        g00, g10 = ln_load(0, 0), ln_load(1, 0)
        ln_final(0, 0, g00)
        ln_final(1, 0, g10)
        g01, g11 = ln_load(0, 1), ln_load(1, 1)
        ln_final(0, 1, g01)
        ln_final(1, 1, g11)


_NC_CACHE = {}


def _get_nc(affine: bool = True):
    if affine not in _NC_CACHE:
        _install_patch()
        _NC_CACHE[affine] = build_nc(affine)
    return _NC_CACHE[affine]


def kernel(**inputs) -> np.ndarray:
    query = np.asarray(inputs["query"], np.float32)  # (L, N, E)
    key = np.asarray(inputs["key"], np.float32)
    Wq = np.asarray(inputs["Wq"], np.float32)        # (E, E)
    bq = np.asarray(inputs["bq"], np.float32)
    Wk = np.asarray(inputs["Wk"], np.float32)
    bk = np.asarray(inputs["bk"], np.float32)
    memory = np.asarray(inputs["memory"], np.float32)  # (MAXL, E)
    ln_w = np.asarray(inputs["ln_w"], np.float32)
    ln_b = np.asarray(inputs["ln_b"], np.float32)

    bf = ml_dtypes.bfloat16
    # (E, N, L) once; per group slice its two batches -> [E, 2L] batch-major
    xqT_all = np.ascontiguousarray(query.transpose(2, 1, 0))
    xkT_all = np.ascontiguousarray(key.transpose(2, 1, 0))
    xqT_g = [
        np.ascontiguousarray(xqT_all[:, 2 * g : 2 * g + 2, :].reshape(E, ROWSC)).astype(bf)
        for g in range(2)
    ]
    xkT_g = [
        np.ascontiguousarray(xkT_all[:, 2 * g : 2 * g + 2, :].reshape(E, ROWSC)).astype(bf)
        for g in range(2)
    ]

    # trivial LN affine (w==1, b==0) is compiled out of the kernel
    affine = not (np.all(ln_w == 1.0) and np.all(ln_b == 0.0))
    nc = _get_nc(affine)
    in_maps = []
    for c in range(NCORES):
        g, p = divmod(c, NGRP)
        sl = slice(p * ESL, (p + 1) * ESL)
        in_maps.append(
            {
                "xqT": xqT_g[g],
                "xkT": xkT_g[g],
                "wqT": np.ascontiguousarray(Wq[sl, :].T).astype(bf),
                "wkT": np.ascontiguousarray(Wk[sl, :].T).astype(bf),
                "bq": np.ascontiguousarray(bq[sl]),
                "bk": np.ascontiguousarray(bk[sl]),
                "memsc": (LAM * memory[:L, sl]).astype(bf),
                "lnw": np.ascontiguousarray(ln_w[sl]),
                "lnb": np.ascontiguousarray(ln_b[sl]),
            }
        )

    res = run_bass_kernel_spmd(nc, in_maps, core_ids=list(range(NCORES)))
    full = np.empty((L, N, E), np.float32)
    for c in range(NCORES):
        g, p = divmod(c, NGRP)
        o = res.results[c]["out"]  # (L, NB, ESL)
        for nl in range(NB):
            full[:, 2 * g + nl, p * ESL : (p + 1) * ESL] = o[:, nl, :]
    return full

